# revision 1
# baseline (speedup 1.0000x reference)
"""Trainium2 Bass kernel for the MFVI second-order CRF message-passing module.

Math (per batch element, per iteration):
    q_sm = softmax(q, axis=-1)                               # over T=256
    Lj   = q_sm @ Tj          (j=1,2)
    Rj   = q_sm @ Tj.T        (j=1,2)
    msg[s] = L1[s-1] + L2[s-2] + R1[s+1] + R2[s+2]           # 0 outside [0,S)
    q    = (unary + msg + start/end-corrections) * mask

Device strategy (8 cores, data-parallel over batch B=32 -> 4/core):
  * Everything stored transposed: [T(2x128 partitions), S(free)] per batch
    element, so the +-1/+-2 sequence shifts become free-dim offsets of the
    matmul's moving operand, accumulated directly into PSUM (msg is never
    materialized wide).
  * softmax over T = partition reduction: ones[128x128] matmul gives the
    column sums broadcast to all partitions in one pass; DVE reciprocal +
    one multiply normalizes.
  * start/end transition scatter-adds are folded on the HOST into a
    corrected unary' (they always land at unmasked positions); the raw
    unary is shipped separately as the softmax-chain seed.
  * matmuls run in float32r (full PE rate); everything else fp32.
"""
import os
import sys

sys.path.insert(0, "/opt/trn_rl_repo")

import numpy as np

import concourse.mybir as mybir
from concourse.bass import Bass
from concourse.tile import TileContext
from concourse import bass_utils

B, S, T = 32, 1024, 256
WINDOW = 2
ITERS = 3
N_CORES = 8
BPC = B // N_CORES          # batch elems per core
NCH = T // 128              # partition chunks of T
HALF = S // 2               # masked half starts here (lengths >= S//2)

# matmul dtype: float32r streams 1 row/cycle (4x faster than float32) at
# ~tf32-ish precision; set MFVI_FP32=1 to force exact-rate fp32.
MM_DT = mybir.dt.float32 if os.environ.get("MFVI_FP32") else mybir.dt.float32r


def _split_sync_waits(nc):
    """walrus in this env accepts at most ONE sync wait per instruction;
    Tile emits several. Move extras onto same-engine NoOps inserted just
    before the offending instruction."""
    ctr = 0
    for f in nc.m.functions:
        for block in f.blocks:
            out = []
            changed = False
            for inst in block.instructions:
                si = inst.sync_info
                waits = list(si.on_wait) if si is not None and si.on_wait else []
                if len(waits) > 1:
                    changed = True
                    for w in waits[:-1]:
                        ctr += 1
                        nop = mybir.InstNoOp(
                            name=f"I-waitsplit-{ctr}",
                            engine=inst.engine, ins=[], outs=[])
                        nop.sync_info = mybir.SyncInfo(on_wait=[w], on_update=[])
                        out.append(nop)
                    si.on_wait = [waits[-1]]
                    inst.sync_info = si
                out.append(inst)
            if changed:
                block.instructions = out
    return nc


def _build():
    f32 = mybir.dt.float32
    nc = Bass(trn_type="TRN2", target_bir_lowering=False, debug=False,
              num_devices=N_CORES)

    u_raw = nc.dram_tensor("u_raw", [BPC, NCH, 128, S], f32,
                           kind="ExternalInput").ap()
    u_corr = nc.dram_tensor("u_corr", [BPC, NCH, 128, S], f32,
                            kind="ExternalInput").ap()
    wmat = nc.dram_tensor("wmat", [128, 4 * NCH * NCH * 128], f32,
                          kind="ExternalInput").ap()
    maskbc = nc.dram_tensor("maskbc", [BPC, 128, HALF], f32,
                            kind="ExternalInput").ap()
    qout = nc.dram_tensor("qout", [BPC, NCH, 128, S], f32,
                          kind="ExternalOutput").ap()

    with TileContext(nc) as tc:
        with tc.tile_pool(name="persist", bufs=1) as pp, \
             tc.tile_pool(name="work", bufs=2) as wp, \
             tc.tile_pool(name="psum", bufs=2, space="PSUM") as psp:

            # ---------- setup ----------
            q = [[pp.tile([128, S], f32, tag=f"q{n}_{c}", name=f"q{n}_{c}") for c in range(NCH)]
                 for n in range(BPC)]
            up = [[pp.tile([128, S], f32, tag=f"up{n}_{c}", name=f"up{n}_{c}") for c in range(NCH)]
                  for n in range(BPC)]
            wst = pp.tile([128, 4 * NCH * NCH * 128], f32, tag="wst", name="wst")

            # DMA order: batch 0 and the weights first so compute starts
            # while the rest of the batch streams in (HBM-bandwidth bound).
            for c in range(NCH):
                nc.sync.dma_start(out=q[0][c], in_=u_raw[0, c])
            nc.sync.dma_start(out=wst, in_=wmat)
            for c in range(NCH):
                nc.sync.dma_start(out=up[0][c], in_=u_corr[0, c])
            for n in range(1, BPC):
                for c in range(NCH):
                    nc.sync.dma_start(out=q[n][c], in_=u_raw[n, c])
                for c in range(NCH):
                    nc.sync.dma_start(out=up[n][c], in_=u_corr[n, c])

            wmm = pp.tile([128, 4 * NCH * NCH * 128], MM_DT, tag="wmm", name="wmm")

            ones_f = pp.tile([128, 128], f32, tag="ones_f", name="ones_f")
            nc.vector.memset(ones_f[:], 1.0)
            ones_m = pp.tile([128, 128], MM_DT, tag="ones_m", name="ones_m")
            nc.vector.tensor_copy(out=ones_m[:], in_=ones_f[:])

            # persistent softmax tiles, padded with WINDOW zero guard
            # columns both sides so shifted conv matmuls never read OOB
            zer = pp.tile([128, WINDOW], f32, tag="zer", name="zer")
            nc.vector.memset(zer[:], 0.0)
            SP = S + 2 * WINDOW
            qsm = [[pp.tile([128, SP], MM_DT, tag=f"qsm{n}_{c}",
                            name=f"qsm{n}_{c}") for c in range(NCH)]
                   for n in range(BPC)]
            for n in range(BPC):
                for c in range(NCH):
                    nc.vector.tensor_copy(out=qsm[n][c][:, 0:WINDOW],
                                          in_=zer[:])
                    nc.vector.tensor_copy(out=qsm[n][c][:, S + WINDOW:],
                                          in_=zer[:])
            nc.vector.tensor_copy(out=wmm[:], in_=wst[:])

            # mask, pre-broadcast on host, second half of S only (first
            # half is always unmasked: lengths >= S/2)
            mb = []
            for n in range(BPC):
                t = pp.tile([128, HALF], f32, tag=f"mb{n}", name=f"mb{n}")
                nc.sync.dma_start(out=t, in_=maskbc[n])
                mb.append(t)

            # ---------- MFVI iterations ----------
            # shift order per S-tile: the first matmul of each accumulation
            # group must cover the full 512-column range of its PSUM tile.
            # mats: 0,1 = left (T1,T2: shift -1,-2); 2,3 = right (T1^T,T2^T:
            # shift +1,+2)
            # PE warm-up: ~4us of dummy matmuls so HAM unthrottles the
            # clock before the first real colsum/conv arrives.
            pwarm = psp.tile([128, S], f32, tag="z", name="pwarm", bufs=1)
            for k in range(20):
                nc.tensor.matmul(pwarm[:, 0:128], ones_m[:], ones_m[:],
                                 start=True, stop=True)

            shifts = [(0, -1), (1, -2), (2, +1), (3, +2)]
            NSTEP = ITERS * BPC

            def emit_exp(step):
                it, n = divmod(step, BPC)
                for c in range(NCH):
                    nc.scalar.activation(
                        out=qsm[n][c][:, WINDOW:S + WINDOW],
                        in_=q[n][c][:],
                        func=mybir.ActivationFunctionType.Exp)

            def emit_zchain(step):
                it, n = divmod(step, BPC)
                pz = psp.tile([128, S], f32, tag="z", name=f"z_{it}_{n}", bufs=1)
                for c in range(NCH):
                    for h in range(2):
                        nc.tensor.matmul(
                            pz[:, h * HALF:(h + 1) * HALF],
                            ones_m[:],
                            qsm[n][c][:, WINDOW + h * HALF:
                                      WINDOW + (h + 1) * HALF],
                            start=(c == 0), stop=(c == NCH - 1))
                rb = wp.tile([128, S], f32, tag="rb", name=f"rb_{it}_{n}")
                nc.vector.reciprocal(rb[:], pz[:])
                for c in range(NCH):
                    nc.vector.tensor_mul(
                        qsm[n][c][:, WINDOW:S + WINDOW],
                        qsm[n][c][:, WINDOW:S + WINDOW], rb[:])

            def emit_conv(step):
                it, n = divmod(step, BPC)
                for st in range(2):
                    s0 = st * HALF
                    pm = [psp.tile([128, HALF], f32, tag=f"m{c}",
                                   name=f"m_{it}_{n}_{st}_{c}", bufs=3)
                          for c in range(NCH)]
                    nmm = len(shifts) * NCH
                    cnt = [0, 0]
                    for (m, d) in shifts:
                        for kc in range(NCH):
                            lo = WINDOW + s0 + d
                            for c in range(NCH):
                                i = (m * NCH + kc) * NCH + c
                                nc.tensor.matmul(
                                    pm[c][:],
                                    wmm[:, i * 128:(i + 1) * 128],
                                    qsm[n][kc][:, lo:lo + HALF],
                                    start=(cnt[c] == 0),
                                    stop=(cnt[c] == nmm - 1))
                                cnt[c] += 1
                    for c in range(NCH):
                        # PSUM evacuation must be DVE (GpSimd cannot touch
                        # PSUM); masks are SBUF-only and go to GpSimd.
                        if it == ITERS - 1:
                            # final iteration: q is dead afterwards, so land
                            # results in scratch tiles (no WAR on q) and ship
                            # each half as soon as it is finished.
                            qf = wp.tile([128, HALF], f32, tag=f"qf{st}_{c}",
                                         name=f"qf_{n}_{st}_{c}", bufs=4)
                            nc.vector.tensor_add(
                                out=qf[:], in0=pm[c][:],
                                in1=up[n][c][:, s0:s0 + HALF])
                            if st == 1:
                                # last batch: keep the whole tail chain on
                                # DVE; earlier batches offload to GpSimd
                                meng = (nc.vector if n == BPC - 1
                                        else nc.gpsimd)
                                meng.tensor_mul(
                                    out=qf[:], in0=qf[:], in1=mb[n][:])
                            nc.sync.dma_start(
                                out=qout[n, c][:, s0:s0 + HALF], in_=qf[:])
                        else:
                            nc.vector.tensor_add(
                                out=q[n][c][:, s0:s0 + HALF],
                                in0=pm[c][:], in1=up[n][c][:, s0:s0 + HALF])
                            if st == 1:
                                nc.gpsimd.tensor_mul(
                                    out=q[n][c][:, HALF:],
                                    in0=q[n][c][:, HALF:], in1=mb[n][:])

            # 1-step software pipeline: Z/softmax of step s+1 is emitted
            # before the conv of step s, so the in-order PE queue can fill
            # the normalize wait of step s with colsum work of step s+1.
            emit_exp(0)
            for step in range(NSTEP + 1):
                if step + 1 < NSTEP:
                    emit_exp(step + 1)
                if step < NSTEP:
                    emit_zchain(step)
                if step >= 1:
                    emit_conv(step - 1)

    _split_sync_waits(nc)
    return nc


_CACHED_NC = None


def _get_nc():
    global _CACHED_NC
    if _CACHED_NC is None:
        _CACHED_NC = _build()
    return _CACHED_NC


def _host_prep(token_feats, unary_score, mask, transitions, start_transitions,
               end_transitions, lengths):
    mask = np.asarray(mask, np.float32)
    unary_score = np.asarray(unary_score, np.float32)
    transitions = np.asarray(transitions, np.float32)
    start_transitions = np.asarray(start_transitions, np.float32)
    end_transitions = np.asarray(end_transitions, np.float32)
    lengths = np.asarray(lengths).astype(np.int64)

    unary = unary_score * mask[..., None]                      # [B,S,T]
    ucorr = unary.copy()
    ucorr[:, 0:WINDOW, :] += start_transitions[None, :, :]
    bidx = np.arange(B)
    for j in range(1, WINDOW + 1):
        ucorr[bidx, lengths - j, :] += end_transitions[j - 1][None, :]

    def to_t(x):  # [B,S,T] -> [B, NCH, 128, S]
        return np.ascontiguousarray(
            x.transpose(0, 2, 1).reshape(B, NCH, 128, S))

    u_rawT = to_t(unary)
    u_corrT = to_t(ucorr)

    # lhsT weight blocks: i = (m*NCH + kc)*NCH + mc
    mats = [transitions[0], transitions[1],
            transitions[0].T, transitions[1].T]
    wblk = np.empty((4 * NCH * NCH, 128, 128), np.float32)
    for m in range(4):
        for kc in range(NCH):
            for mc in range(NCH):
                wblk[(m * NCH + kc) * NCH + mc] = \
                    mats[m][kc * 128:(kc + 1) * 128, mc * 128:(mc + 1) * 128]
    # device layout: [128 partitions(k-within-chunk), 16 blocks x 128]
    wblk = np.ascontiguousarray(
        wblk.transpose(1, 0, 2).reshape(128, 4 * NCH * NCH * 128))
    return u_rawT, u_corrT, wblk, mask


def kernel(token_feats, unary_score, mask, transitions, start_transitions,
           end_transitions, lengths):
    u_rawT, u_corrT, wblk, maskf = _host_prep(
        token_feats, unary_score, mask, transitions, start_transitions,
        end_transitions, lengths)

    in_maps = []
    for core in range(N_CORES):
        sl = slice(core * BPC, (core + 1) * BPC)
        in_maps.append({
            "u_raw": np.ascontiguousarray(u_rawT[sl]),
            "u_corr": np.ascontiguousarray(u_corrT[sl]),
            "wmat": wblk,
            "maskbc": np.ascontiguousarray(
                np.broadcast_to(maskf[sl, None, HALF:],
                                (BPC, 128, HALF)).astype(np.float32)),
        })

    nc = _get_nc()
    res = bass_utils.run_bass_kernel_spmd(nc, in_maps,
                                          core_ids=list(range(N_CORES)))
    qT = np.concatenate([res.results[c]["qout"] for c in range(N_CORES)],
                        axis=0)                                # [B,NCH,128,S]
    q = qT.reshape(B, T, S).transpose(0, 2, 1)                 # [B,S,T]
    return np.ascontiguousarray(q.astype(np.float32))

